# revision 3
# baseline (speedup 1.0000x reference)
# GCNConv (dense adjacency, symmetric normalization) on 8 trn2 NeuronCores.
#
#   out = D^{-1/2} A D^{-1/2} (x @ W) + bias,   deg = A.sum(axis=1)
#
# Strategy (row-shard, 1D graph partition):
#   - core c owns output rows [1024c, 1024(c+1)); its shard of A is passed
#     host-side pre-transposed (adjT_c = A[rows_c, :].T, shape [8192, 1024])
#     so that the contraction index j (columns of A) lands on the SBUF
#     partition axis with plain contiguous DMAs - no on-chip transposes of
#     the 32MB shard are needed.
#   - The shard is DMA-cast fp32->bf16 on load and kept SBUF-resident
#     (16MB), so HBM traffic is one pass over A (memory roofline).
#   - deg (row sums of A) = ones^T @ adjT via the tensor engine, accumulated
#     over all 64 j-blocks; a tiny (4KB/core) AllGather distributes deg.
#   - dinv = 1/sqrt(deg) via ACT sqrt + DVE reciprocal + one Newton step.
#   - h = x @ W computed per-core from xT (host-transposed x, replicated);
#     H' = dinv * h cast to bf16 feeds the SpMM as the stationary operand:
#        outT[d, i] += sum_j H'[j, d] * adjT[j, i]
#     accumulated in PSUM over the 64 j-blocks, then transposed back,
#     scaled by local dinv rows, bias added, and DMA'd out.

import numpy as np

N = 8192
D = 128
NCORES = 8
P = 128


def _build(n=N, d=D, ncores=NCORES):
    from contextlib import ExitStack

    import concourse.bacc as bacc
    import concourse.bass as bass
    import concourse.masks as masks
    import concourse.mybir as mybir
    import concourse.tile as tile

    f32 = mybir.dt.float32
    bf16 = mybir.dt.bfloat16
    mult = mybir.AluOpType.mult
    add = mybir.AluOpType.add

    nb = n // P  # number of j-blocks (64)
    rpc = n // ncores  # rows per core (1024)
    lb = rpc // P  # local row tiles (8)
    nhalf = min(512, rpc)  # moving free-dim per matmul (PSUM bank limit)
    halves = rpc // nhalf
    xch = min(2048, n)  # x chunk width (free dim)

    nc = bacc.Bacc("TRN2", target_bir_lowering=False, debug=False, num_devices=ncores)

    adjT = nc.dram_tensor("adjT", [n, rpc], f32, kind="ExternalInput")
    xT = nc.dram_tensor("xT", [d, n], f32, kind="ExternalInput")
    w = nc.dram_tensor("w", [d, d], f32, kind="ExternalInput")
    bias = nc.dram_tensor("bias", [d], f32, kind="ExternalInput")
    out = nc.dram_tensor("out", [rpc, d], f32, kind="ExternalOutput")

    with tile.TileContext(nc) as tc, ExitStack() as ctx:
        singles = ctx.enter_context(tc.tile_pool(name="singles", bufs=1))
        dram = ctx.enter_context(tc.tile_pool(name="dram", bufs=1, space="DRAM"))
        atp = ctx.enter_context(tc.tile_pool(name="atp", bufs=1))
        xcp = ctx.enter_context(tc.tile_pool(name="xcp", bufs=2))
        hbp = ctx.enter_context(tc.tile_pool(name="hbp", bufs=4))
        psdeg = ctx.enter_context(tc.tile_pool(name="psdeg", bufs=1, space="PSUM"))
        psh = ctx.enter_context(tc.tile_pool(name="psh", bufs=2, space="PSUM"))
        psout = ctx.enter_context(tc.tile_pool(name="psout", bufs=1, space="PSUM"))
        psmisc = ctx.enter_context(tc.tile_pool(name="psmisc", bufs=2, space="PSUM"))

        # ---- constants ----
        ident = singles.tile([P, P], f32)
        masks.make_identity(nc, ident[:])
        ones_bf = singles.tile([P, 1], bf16)
        nc.gpsimd.memset(ones_bf[:], 1.0)
        ones_row = singles.tile([1, P], f32)
        nc.gpsimd.memset(ones_row[:], 1.0)
        w_sb = singles.tile([d, d], f32)
        nc.sync.dma_start(w_sb[:], w[:, :])
        bias_row = singles.tile([1, d], f32)
        nc.sync.dma_start(bias_row[:], bias[:])
        # broadcast bias across partitions: ones[1,P].T @ bias[1,d]
        bias_mat = singles.tile([P, d], f32)
        bm_ps = psmisc.tile([P, d], f32, tag="misc")
        nc.tensor.matmul(bm_ps[:], ones_row[:], bias_row[:])
        nc.vector.tensor_copy(bias_mat[:], bm_ps[:])

        # ---- big SBUF residents ----
        AT = atp.tile([P, nb * rpc], bf16)  # adjT, bf16, [j-part, (b i)]
        Hp = singles.tile([P, nb * d], f32)  # h = x@W fp32, [j-part, (b d)]

        # ---- adjT load (cast fp32->bf16) + degree accumulation on PE ----
        deg_ps = [psdeg.tile([1, nhalf], f32, name=f"deg_ps{h}") for h in range(halves)]
        for b in range(nb):
            nc.gpsimd.dma_start(AT[:, b * rpc : (b + 1) * rpc], adjT[b * P : (b + 1) * P, :])
            for h in range(halves):
                nc.tensor.matmul(
                    deg_ps[h][:],
                    ones_bf[:],
                    AT[:, b * rpc + h * nhalf : b * rpc + (h + 1) * nhalf],
                    start=(b == 0),
                    stop=(b == nb - 1),
                )

        # ---- h = x @ W  (lhsT = xT block [din, j], rhs = W [din, dout]) ----
        for c0 in range(0, n, xch):
            xc = xcp.tile([d, xch], f32)
            nc.sync.dma_start(xc[:], xT[:, c0 : c0 + xch])
            for bb in range(xch // P):
                b = c0 // P + bb
                h_ps = psh.tile([P, d], f32)
                nc.tensor.matmul(h_ps[:], xc[:, bb * P : (bb + 1) * P], w_sb[:])
                nc.scalar.copy(Hp[:, b * d : (b + 1) * d], h_ps[:])

        # ---- local deg -> DRAM -> AllGather -> global deg ----
        deg_sb = singles.tile([1, rpc], f32)
        for h in range(halves):
            nc.vector.tensor_copy(deg_sb[:, h * nhalf : (h + 1) * nhalf], deg_ps[h][:])
        ag_in = dram.tile([rpc], f32)
        ag_out = dram.tile([n], f32, addr_space="Shared")
        nc.sync.dma_start(ag_in[:], deg_sb[:1, :])
        nc.gpsimd.collective_compute(
            "AllGather",
            mybir.AluOpType.bypass,
            replica_groups=[list(range(ncores))],
            ins=[ag_in.opt()],
            outs=[ag_out.opt()],
        )
        degg_sb = singles.tile([nb, P], f32)
        nc.sync.dma_start(degg_sb[:], ag_out[:])

        def rsqrt_newton(dst, deg_psum, width, scratch_tag):
            # dst = 1/sqrt(deg) with one Newton step (ACT sqrt is low-precision)
            dgc = singles.tile([P, width], f32, name=f"dgc_{scratch_tag}")
            nc.vector.tensor_copy(dgc[:], deg_psum[:])
            sq = singles.tile([P, width], f32, name=f"sq_{scratch_tag}")
            nc.scalar.sqrt(sq[:], deg_psum[:])
            r0 = singles.tile([P, width], f32, name=f"r0_{scratch_tag}")
            nc.vector.reciprocal(r0[:], sq[:])
            t0 = singles.tile([P, width], f32, name=f"t0_{scratch_tag}")
            nc.vector.tensor_mul(t0[:], r0[:], r0[:])
            nc.vector.tensor_mul(t0[:], t0[:], dgc[:])
            nc.vector.tensor_scalar(t0[:], t0[:], -0.5, 1.5, mult, add)
            nc.vector.tensor_mul(dst[:], t0[:], r0[:])

        # global dinv in [p, b] layout: transpose degg [nb, P] -> [P, nb]
        dgt_ps = psmisc.tile([P, nb], f32, tag="misc")
        nc.tensor.transpose(dgt_ps[:], degg_sb[:], ident[:nb, :nb])
        dinvg = singles.tile([P, nb], f32)
        rsqrt_newton(dinvg, dgt_ps, nb, "g")

        # local dinv in [p, r] layout (for the output rows of this core)
        dloc_ps = psmisc.tile([P, lb], f32, tag="misc")
        for r in range(lb):
            nc.tensor.transpose(
                dloc_ps[:, r : r + 1], deg_sb[:1, r * P : (r + 1) * P], ident[:1, :1]
            )
        dinvl = singles.tile([P, lb], f32)
        rsqrt_newton(dinvl, dloc_ps, lb, "l")

        # ---- main SpMM: outT[d, i] += sum_j H'[j, d] * adjT[j, i] ----
        out_ps = [psout.tile([P, nhalf], f32, name=f"out_ps{h}") for h in range(halves)]
        for b in range(nb):
            hb = hbp.tile([P, d], bf16)
            nc.vector.tensor_scalar(hb[:], Hp[:, b * d : (b + 1) * d], dinvg[:, b : b + 1], None, mult)
            for h in range(halves):
                nc.tensor.matmul(
                    out_ps[h][:],
                    hb[:],
                    AT[:, b * rpc + h * nhalf : b * rpc + (h + 1) * nhalf],
                    start=(b == 0),
                    stop=(b == nb - 1),
                )

        # ---- finalize: transpose back, scale by dinv rows, add bias ----
        outT_sb = singles.tile([P, rpc], f32)
        for h in range(halves):
            nc.scalar.copy(outT_sb[:, h * nhalf : (h + 1) * nhalf], out_ps[h][:])
        out_sb = singles.tile([P, lb * d], f32)
        for r in range(lb):
            ob_ps = psmisc.tile([P, d], f32, tag="misc")
            nc.tensor.transpose(ob_ps[:], outT_sb[:, r * P : (r + 1) * P], ident[:])
            nc.vector.tensor_scalar(out_sb[:, r * d : (r + 1) * d], ob_ps[:], dinvl[:, r : r + 1], None, mult)
            nc.vector.tensor_add(out_sb[:, r * d : (r + 1) * d], out_sb[:, r * d : (r + 1) * d], bias_mat[:])
        nc.sync.dma_start(
            out.ap().rearrange("(r p) d -> p r d", p=P),
            out_sb[:].rearrange("p (r d) -> p r d", d=d),
        )

    nc.compile()
    return nc


_NC_CACHE = {}


def _get_nc(n=N, d=D, ncores=NCORES):
    key = (n, d, ncores)
    if key not in _NC_CACHE:
        _NC_CACHE[key] = _build(n, d, ncores)
    return _NC_CACHE[key]


def run(x, adj, weight, bias, n=N, d=D, ncores=NCORES, trace=False):
    from concourse import bass_utils

    x = np.ascontiguousarray(np.asarray(x, dtype=np.float32))
    adj = np.ascontiguousarray(np.asarray(adj, dtype=np.float32))
    weight = np.ascontiguousarray(np.asarray(weight, dtype=np.float32))
    bias = np.ascontiguousarray(np.asarray(bias, dtype=np.float32))

    rpc = n // ncores
    xT = np.ascontiguousarray(x.T)
    in_maps = []
    for c in range(ncores):
        adjT_c = np.ascontiguousarray(adj[c * rpc : (c + 1) * rpc, :].T)
        in_maps.append({"adjT": adjT_c, "xT": xT, "w": weight, "bias": bias})

    nc = _get_nc(n, d, ncores)
    res = bass_utils.run_bass_kernel_spmd(
        nc, in_maps, core_ids=list(range(ncores)), trace=trace
    )
    out = np.concatenate([r["out"] for r in res.results], axis=0)
    return out, res


def kernel(x, adj, weight, bias):
    out, _ = run(x, adj, weight, bias)
    return out
